# revision 10
# baseline (speedup 1.0000x reference)
"""AttentionPool2d kernel for 8x TRN2 NeuronCores.

Math (per batch element):
  x_aug = [mean(x) | x] + pos_emb                          # [E, T], T=1025
  q0    = W_q @ x_aug[:,0] + b_q                           # only token 0 of the
                                                           # output is used!
  u_h   = (q0_h^T W_k,h) / 8 ; c_h = (q0_h . b_k,h) / 8    # per head h
  logits[h,s] = u_h . x_aug[:,s] + c_h
  w     = softmax_s(logits)                                # [H, T]
  z_h   = x_aug @ w_h                                      # [E] per head
  att_h = W_v,h @ z_h + b_v,h                              # [64] per head
  out   = W_c @ att + b_c                                  # [E]

Sharding: data-parallel, 2 batch elements per core, no collectives.
"""

import sys
import numpy as np

if "/opt/trn_rl_repo" not in sys.path:
    sys.path.insert(0, "/opt/trn_rl_repo")

E = 1024
T = 1025        # 1024 tokens + 1 mean token
TP = 1152       # T padded to 9*128 for transposes
H = 16
B = 16
NCORES = 8
BPC = B // NCORES   # batches per core = 2
P = 128
NCH = E // P        # 8 chunks of 128 channels

_CACHE = {}


def _build():
    import concourse.bass as bass
    import concourse.mybir as mybir
    import concourse.tile as tile
    from concourse import bacc
    from concourse.masks import make_identity

    fp32 = mybir.dt.float32
    AX = mybir.AxisListType.X
    Exp = mybir.ActivationFunctionType.Exp

    nc = bacc.Bacc("TRN2", target_bir_lowering=False)
    x_in = nc.declare_dram_parameter("x", [BPC, E, T - 1], fp32, isOutput=False)
    pos = nc.declare_dram_parameter("pos_emb", [E, T], fp32, isOutput=False)
    wqkv = nc.declare_dram_parameter("w_qkv", [3 * E, E], fp32, isOutput=False)
    bqkv = nc.declare_dram_parameter("b_qkv", [3 * E], fp32, isOutput=False)
    wc_d = nc.declare_dram_parameter("w_c", [E, E], fp32, isOutput=False)
    bc_d = nc.declare_dram_parameter("b_c", [E], fp32, isOutput=False)
    out_d = nc.declare_dram_parameter("out", [BPC, E], fp32, isOutput=True)

    with tile.TileContext(nc) as tc:
        with (
            tc.tile_pool(name="main", bufs=1) as mp,
            tc.tile_pool(name="wbig", bufs=2) as wp,
            tc.tile_pool(name="stage", bufs=3) as sp,
            tc.tile_pool(name="lgp", bufs=2) as lp,
            tc.tile_pool(name="psA", bufs=2, space="PSUM") as psA,
            tc.tile_pool(name="psU", bufs=1, space="PSUM") as psU,
            tc.tile_pool(name="psL", bufs=2, space="PSUM") as psL,
            tc.tile_pool(name="psT", bufs=2, space="PSUM") as psT,
        ):
            ident = mp.tile([P, P], fp32)
            make_identity(nc, ident)

            # ---- persistent SBUF tensors -------------------------------
            xaug = [mp.tile([P, NCH, TP], fp32, tag=f"xaug{b}", name=f"xaug{b}") for b in range(BPC)]
            # big-tag chain: wqT -> wk -> wvT -> wcT share 2 slots
            BIGSHAPE = [P, NCH, T]      # 1025 >= 1024, max of all users
            # pos_emb via SWDGE (1 sem per DMA) keeps consumer wait counts low
            pos_sb = mp.tile([P, NCH, T], fp32)
            for j in range(NCH):
                nc.gpsimd.dma_start(pos_sb[:, j, :], pos[P * j:P * (j + 1), :])

            # biases
            bq_sb = mp.tile([P, NCH], fp32)
            bk_sb = mp.tile([P, NCH], fp32)
            bv_sb = mp.tile([P, NCH], fp32)
            bc_sb = mp.tile([P, NCH], fp32)
            nc.sync.dma_start(bq_sb, bqkv[0:E].rearrange("(j p) -> p j", p=P))
            nc.sync.dma_start(bk_sb, bqkv[E:2 * E].rearrange("(j p) -> p j", p=P))
            nc.sync.dma_start(bv_sb, bqkv[2 * E:3 * E].rearrange("(j p) -> p j", p=P))
            nc.sync.dma_start(bc_sb, bc_d.rearrange("(j p) -> p j", p=P))

            # ---- load x, build x_aug = [mean|x] + pos ------------------
            x0t = mp.tile([P, NCH, BPC], fp32)          # x_aug[:, 0] both batches
            for b in range(BPC):
                for j in range(NCH):
                    nc.sync.dma_start(
                        xaug[b][:, j, 1:T], x_in[b, P * j:P * (j + 1), :]
                    )
                    mtmp = sp.tile([P, 1], fp32, tag="mean")
                    nc.vector.reduce_sum(mtmp, xaug[b][:, j, 1:T], axis=AX)
                    nc.scalar.mul(mtmp, mtmp, 1.0 / (T - 1))
                    nc.vector.tensor_add(
                        x0t[:, j, b:b + 1], mtmp, pos_sb[:, j, 0:1]
                    )
                    nc.vector.tensor_copy(xaug[b][:, j, 0:1], x0t[:, j, b:b + 1])
                    nc.vector.tensor_add(
                        xaug[b][:, j, 1:T], xaug[b][:, j, 1:T], pos_sb[:, j, 1:T]
                    )
                    # zero the transpose pad so garbage never enters PE
                    nc.vector.memset(xaug[b][:, j, T:TP], 0.0)

            # ---- W_q^T (big-tag slot 2) --------------------------------
            def build_wT(dst, src_ap):
                """dst[c_p, ci, o] = src[o, c] transposed, src is [E, E] DRAM."""
                for j in range(NCH):            # chunk of output rows o
                    wnat = sp.tile([P, E], fp32, tag="wnat")
                    nc.sync.dma_start(wnat, src_ap[P * j:P * (j + 1), :])
                    for i in range(NCH):        # chunk of columns c
                        pt = psT.tile([P, P], fp32, tag="tp")
                        nc.tensor.transpose(pt, wnat[:, P * i:P * (i + 1)], ident)
                        nc.vector.tensor_copy(dst[:, i, P * j:P * (j + 1)], pt)

            wqT = wp.tile(BIGSHAPE, fp32, tag="big", name="wqT")
            build_wT(wqT, wqkv[0:E])

            # ---- q0 = W_q @ x0 + b_q  ->  [o, b] tiles -----------------
            q0sb = mp.tile([P, NCH, BPC], fp32)
            for j in range(NCH):
                pq = psA.tile([P, BPC], fp32, tag="sm")
                for i in range(NCH):
                    nc.tensor.matmul(
                        pq, wqT[:, i, P * j:P * (j + 1)], x0t[:, i, :],
                        start=(i == 0), stop=(i == NCH - 1),
                    )
                nc.vector.tensor_add(
                    q0sb[:, j, :], pq, bq_sb[:, j, None].to_broadcast((P, BPC))
                )

            # blockdiag q0: col m = b*16+h holds q0 of head h, batch b
            q0bd = mp.tile([P, NCH, BPC * H], fp32)
            nc.vector.memset(q0bd, 0.0)
            for j in range(NCH):
                for b in range(BPC):
                    nc.vector.tensor_copy(
                        q0bd[0:64, j, b * H + 2 * j:b * H + 2 * j + 1],
                        q0sb[0:64, j, b:b + 1],
                    )
                    nc.vector.tensor_copy(
                        q0bd[64:128, j, b * H + 2 * j + 1:b * H + 2 * j + 2],
                        q0sb[64:128, j, b:b + 1],
                    )

            # ---- wk native (big-tag slot reusing pos) ------------------
            wk = wp.tile(BIGSHAPE, fp32, tag="big", name="wk")
            for j in range(NCH):
                nc.sync.dma_start(wk[:, j, :E], wqkv[E + P * j:E + P * (j + 1), :])

            # ---- u = (q0bd^T @ W_k) / 8, also c = (q0bd^T @ b_k) / 8 ---
            pu = psU.tile([BPC * H, E], fp32, tag="u")
            for j in range(NCH):
                for s in range(2):  # fp32 moving operand max 512
                    nc.tensor.matmul(
                        pu[:, 512 * s:512 * (s + 1)],
                        q0bd[:, j, :], wk[:, j, 512 * s:512 * (s + 1)],
                        start=(j == 0), stop=(j == NCH - 1),
                    )
            usb = mp.tile([BPC * H, E], fp32)
            nc.scalar.mul(usb, pu, 0.125)

            cv = mp.tile([H, BPC], fp32)
            for b in range(BPC):
                pc = psA.tile([H, 1], fp32, tag="sm")
                for j in range(NCH):
                    nc.tensor.matmul(
                        pc, q0bd[:, j, b * H:(b + 1) * H], bk_sb[:, j, None],
                        start=(j == 0), stop=(j == NCH - 1),
                    )
                nc.scalar.mul(cv[:, b:b + 1], pc, 0.125)

            # uT[c_p, ci, m] = usb[m, c]
            uT = mp.tile([P, NCH, BPC * H], fp32)
            for i in range(NCH):
                pt = psT.tile([P, BPC * H], fp32, tag="tp")
                nc.tensor.transpose(pt, usb[:, P * i:P * (i + 1)], ident[:BPC * H, :BPC * H])
                nc.vector.tensor_copy(uT[:, i, :], pt)

            # ---- logits, softmax, wT ----------------------------------
            SEGS = [(0, 512), (512, 512), (1024, 1)]
            wTt = []
            for b in range(BPC):
                lg = lp.tile([H, TP], fp32, tag="lg")
                for (s0, sl) in SEGS:
                    pl = psL.tile([H, 512], fp32, tag="lg")
                    for i in range(NCH):
                        nc.tensor.matmul(
                            pl[:, :sl], uT[:, i, b * H:(b + 1) * H],
                            xaug[b][:, i, s0:s0 + sl],
                            start=(i == 0), stop=(i == NCH - 1),
                        )
                    nc.vector.tensor_add(
                        lg[:, s0:s0 + sl], pl[:, :sl],
                        cv[:, b:b + 1].to_broadcast((H, sl)),
                    )
                nc.vector.memset(lg[:, T:TP], 0.0)
                mx = sp.tile([H, 1], fp32, tag="mx")
                nc.vector.reduce_max(mx, lg[:, :T], axis=AX)
                nmx = sp.tile([H, 1], fp32, tag="nmx")
                nc.scalar.mul(nmx, mx, -1.0)
                nc.scalar.activation(lg[:, :T], lg[:, :T], Exp, bias=nmx, scale=1.0)
                ssum = sp.tile([H, 1], fp32, tag="ssum")
                nc.vector.reduce_sum(ssum, lg[:, :T], axis=AX)
                rs = sp.tile([H, 1], fp32, tag="rs")
                nc.vector.reciprocal(rs, ssum)
                nc.vector.tensor_mul(lg[:, :T], lg[:, :T], rs.to_broadcast((H, T)))

                wT_b = mp.tile([P, 9, H], fp32, tag=f"wT{b}", name=f"wT{b}")
                for k in range(9):
                    pt = psT.tile([P, H], fp32, tag="tp")
                    nc.tensor.transpose(pt, lg[:, P * k:P * (k + 1)], ident[:H, :H])
                    nc.vector.tensor_copy(wT_b[:, k, :], pt)
                wTt.append(wT_b)

            # ---- z^T[c, b*16+h] via on-the-fly x_aug transposes --------
            zT = mp.tile([P, NCH, BPC * H], fp32)
            for b in range(BPC):
                for cj in range(NCH):
                    pz = psA.tile([P, H], fp32, tag="sm")
                    for sk in range(9):
                        pt = psT.tile([P, P], fp32, tag="tp")
                        nc.tensor.transpose(
                            pt, xaug[b][:, cj, P * sk:P * (sk + 1)], ident
                        )
                        stg = sp.tile([P, P], fp32, tag="stg")
                        nc.vector.tensor_copy(stg, pt)
                        nc.tensor.matmul(
                            pz, stg, wTt[b][:, sk, :],
                            start=(sk == 0), stop=(sk == 8),
                        )
                    nc.vector.tensor_copy(zT[:, cj, b * H:(b + 1) * H], pz)

            # ---- W_v^T then att ---------------------------------------
            wvT = wp.tile(BIGSHAPE, fp32, tag="big", name="wvT")
            build_wT(wvT, wqkv[2 * E:3 * E])

            # head-select masks: mask_j[p, h] = 1 iff h == 2j + p//64
            masks = mp.tile([P, NCH, H], fp32)
            nc.vector.memset(masks, 0.0)
            for j in range(NCH):
                nc.vector.memset(masks[0:64, j, 2 * j:2 * j + 1], 1.0)
                nc.vector.memset(masks[64:128, j, 2 * j + 1:2 * j + 2], 1.0)

            attsel = mp.tile([P, NCH, BPC], fp32)
            for oj in range(NCH):
                pa = psA.tile([P, BPC * H], fp32, tag="sm")
                for ci in range(NCH):
                    nc.tensor.matmul(
                        pa, wvT[:, ci, P * oj:P * (oj + 1)], zT[:, ci, :],
                        start=(ci == 0), stop=(ci == NCH - 1),
                    )
                tmp = sp.tile([P, BPC, H], fp32, tag="asel")
                nc.vector.tensor_mul(
                    tmp, pa.rearrange("p (b h) -> p b h", b=BPC),
                    masks[:, oj, None, :].to_broadcast((P, BPC, H)),
                )
                nc.vector.reduce_sum(attsel[:, oj, :], tmp, axis=AX)
                nc.vector.tensor_add(
                    attsel[:, oj, :], attsel[:, oj, :],
                    bv_sb[:, oj, None].to_broadcast((P, BPC)),
                )

            # ---- W_c^T then out ---------------------------------------
            wcT = wp.tile(BIGSHAPE, fp32, tag="big", name="wcT")
            build_wT(wcT, wc_d)

            osb = mp.tile([P, NCH, BPC], fp32)
            for oj in range(NCH):
                po = psA.tile([P, BPC], fp32, tag="sm")
                for ci in range(NCH):
                    nc.tensor.matmul(
                        po, wcT[:, ci, P * oj:P * (oj + 1)], attsel[:, ci, :],
                        start=(ci == 0), stop=(ci == NCH - 1),
                    )
                nc.vector.tensor_add(
                    osb[:, oj, :], po, bc_sb[:, oj, None].to_broadcast((P, BPC))
                )
            for oj in range(NCH):
                nc.sync.dma_start(
                    out_d[:, P * oj:P * (oj + 1)].rearrange("b p -> p b"),
                    osb[:, oj, :],
                )

    nc.finalize()
    return nc


def get_nc():
    if "nc" not in _CACHE:
        _CACHE["nc"] = _build()
    return _CACHE["nc"]


def kernel(x, pos_emb, w_qkv, b_qkv, w_c, b_c):
    from concourse.bass_utils import run_bass_kernel_spmd

    nc = get_nc()
    x = np.ascontiguousarray(x, dtype=np.float32)
    in_maps = [
        {
            "x": x[BPC * j:BPC * (j + 1)],
            "pos_emb": np.asarray(pos_emb, dtype=np.float32),
            "w_qkv": np.asarray(w_qkv, dtype=np.float32),
            "b_qkv": np.asarray(b_qkv, dtype=np.float32),
            "w_c": np.asarray(w_c, dtype=np.float32),
            "b_c": np.asarray(b_c, dtype=np.float32),
        }
        for j in range(NCORES)
    ]
    res = run_bass_kernel_spmd(nc, in_maps, list(range(NCORES)))
    return np.concatenate(
        [res.results[j]["out"] for j in range(NCORES)], axis=0
    ).astype(np.float32)


# revision 25
# speedup vs baseline: 31039.4127x; 31039.4127x over previous
"""AttentionPool2d kernel for 8x TRN2 NeuronCores.

Math (per batch element):
  x_aug = [mean(x) | x] + pos_emb                          # [E, T], T=1025
  q0    = W_q @ x_aug[:,0] + b_q                           # only token 0 of the
                                                           # output is used!
  u_h   = (q0_h^T W_k,h) / 8 ; c_h = (q0_h . b_k,h) / 8    # per head h
  logits[h,s] = u_h . x_aug[:,s] + c_h
  w     = softmax_s(logits)                                # [H, T]
  z_h   = x_aug @ w_h                                      # [E] per head
  att_h = W_v,h @ z_h + b_v,h                              # [64] per head
  out   = W_c @ att + b_c                                  # [E]

Sharding: data-parallel, 2 batch elements per core, no collectives.
Large contractions run as float32r matmuls (full PE rate at N>=256).
"""

import sys
import numpy as np

if "/opt/trn_rl_repo" not in sys.path:
    sys.path.insert(0, "/opt/trn_rl_repo")

E = 1024
T = 1025        # 1024 tokens + 1 mean token
TP = 1152       # T padded to 9*128 for transposes
H = 16
B = 16
NCORES = 8
BPC = B // NCORES   # batches per core = 2
P = 128
NCH = E // P        # 8 chunks of 128 channels

_CACHE = {}


def _build():
    import concourse.bass as bass
    import concourse.mybir as mybir
    import concourse.tile as tile
    from concourse import bacc
    from concourse.masks import make_identity

    fp32 = mybir.dt.float32
    f32r = mybir.dt.float32r
    AX = mybir.AxisListType.X
    Exp = mybir.ActivationFunctionType.Exp

    nc = bacc.Bacc("TRN2", target_bir_lowering=False)
    x_in = nc.declare_dram_parameter("x", [BPC, E, T - 1], fp32, isOutput=False)
    pos = nc.declare_dram_parameter("pos_emb", [E, T], fp32, isOutput=False)
    wqkv = nc.declare_dram_parameter("w_qkv", [3 * E, E], fp32, isOutput=False)
    bqkv = nc.declare_dram_parameter("b_qkv", [3 * E], fp32, isOutput=False)
    wc_d = nc.declare_dram_parameter("w_c", [E, E], fp32, isOutput=False)
    bc_d = nc.declare_dram_parameter("b_c", [E], fp32, isOutput=False)
    out_d = nc.declare_dram_parameter("out", [BPC, E], fp32, isOutput=True)

    with tile.TileContext(nc) as tc:
        with (
            tc.tile_pool(name="main", bufs=1) as mp,
            tc.tile_pool(name="wbig", bufs=3) as wp,
            tc.tile_pool(name="stage", bufs=3) as sp,
            tc.tile_pool(name="small", bufs=3) as smp,
            tc.tile_pool(name="lgp", bufs=1) as lp,
            tc.tile_pool(name="psA", bufs=2, space="PSUM") as psA,
            tc.tile_pool(name="psU", bufs=1, space="PSUM") as psU,
            tc.tile_pool(name="psL", bufs=2, space="PSUM") as psL,
            tc.tile_pool(name="psT", bufs=2, space="PSUM") as psT,
        ):
            ident = mp.tile([P, P], fp32)
            make_identity(nc, ident)
            identr = mp.tile([P, P], f32r)
            nc.vector.tensor_copy(identr, ident)
            zpad = mp.tile([P, TP - T], fp32)
            nc.vector.memset(zpad, 0.0)

            def copyback(idx, dst, src):
                # ACT copy is ~4x slower than DVE: give it a 1/3 share
                if idx % 3 == 2:
                    nc.scalar.copy(dst, src)
                else:
                    nc.vector.tensor_copy(dst, src)

            # ---- persistent SBUF tensors -------------------------------
            xaug = [mp.tile([P, NCH, TP], f32r, tag=f"xaug{b}", name=f"xaug{b}")
                    for b in range(BPC)]
            # shared 3-slot ring: pos_sb -> wstage/wT tensors (32.9KB each)
            BIGSHAPE = [P, NCH, T]
            pos_sb = wp.tile(BIGSHAPE, fp32, tag="big", name="pos_sb")
            nc.sync.dma_start(pos_sb[:, :, :T], pos.rearrange("(o p) t -> p o t", p=P))

            bq_sb = mp.tile([P, NCH], fp32)
            bk_sb = mp.tile([P, NCH], fp32)
            bv_sb = mp.tile([P, NCH], fp32)
            bc_sb = mp.tile([P, NCH], fp32)
            nc.sync.dma_start(bq_sb, bqkv[0:E].rearrange("(j p) -> p j", p=P))
            nc.sync.dma_start(bk_sb, bqkv[E:2 * E].rearrange("(j p) -> p j", p=P))
            nc.sync.dma_start(bv_sb, bqkv[2 * E:3 * E].rearrange("(j p) -> p j", p=P))
            nc.sync.dma_start(bc_sb, bc_d.rearrange("(j p) -> p j", p=P))

            # ---- load x, build x_aug = [mean|x] + pos ------------------
            x0t = mp.tile([P, NCH, BPC], fp32)          # x_aug[:, 0] both batches
            for b in range(BPC):
                for j in range(NCH):
                    # raw x staged so xaug's only producers are rounding
                    # compute ops (FP32r verifier requirement)
                    stgx = sp.tile([P, E], fp32, tag="stq", name=f"stgx_{b}_{j}")[:, :T - 1]
                    nc.sync.dma_start(stgx, x_in[b, P * j:P * (j + 1), :])
                    mtmp = smp.tile([P, 1], fp32, tag="mean")
                    junk = sp.tile([P, E], fp32, tag="stq", name=f"junk_{b}_{j}")[:, :T - 1]
                    nc.scalar.activation(
                        junk, stgx,
                        mybir.ActivationFunctionType.Copy, accum_out=mtmp,
                    )
                    nc.scalar.mul(mtmp, mtmp, 1.0 / (T - 1))
                    nc.vector.tensor_add(
                        x0t[:, j, b:b + 1], mtmp, pos_sb[:, j, 0:1]
                    )
                    nc.vector.tensor_copy(xaug[b][:, j, 0:1], x0t[:, j, b:b + 1])
                    nc.gpsimd.tensor_tensor(
                        xaug[b][:, j, 1:T], stgx,
                        pos_sb[:, j, 1:T], mybir.AluOpType.add,
                    )
                    nc.vector.tensor_copy(xaug[b][:, j, T:TP], zpad)

            # ---- W^T builders (PE transpose, paired copyback) ----------
            def build_wT(dst, src_ap, tagname):
                ws = wp.tile(BIGSHAPE, fp32, tag="big", name=f"ws_{tagname}")
                nc.sync.dma_start(
                    ws[:, :, :E], src_ap.rearrange("(j p) c -> p j c", p=P)
                )
                cb = 0
                for j in range(NCH):            # chunk of src rows o
                    for i in range(0, NCH, 2):  # chunks of src cols c, paired
                        pt = psT.tile([P, 2, P], fp32, tag="tp2")
                        nc.tensor.transpose(
                            pt[:, 0, :], ws[:, j, P * i:P * (i + 1)], ident
                        )
                        nc.tensor.transpose(
                            pt[:, 1, :], ws[:, j, P * (i + 1):P * (i + 2)], ident
                        )
                        copyback(cb, dst[:, i:i + 2, P * j:P * (j + 1)], pt)
                        cb += 1

            wqT = wp.tile(BIGSHAPE, fp32, tag="big", name="wqT")
            build_wT(wqT, wqkv[0:E], "q")

            # ---- q0 = W_q @ x0 + b_q  ->  [o, b] tiles -----------------
            q0sb = mp.tile([P, NCH, BPC], fp32)
            for j in range(NCH):
                pq = psA.tile([P, BPC], fp32, tag="sm")
                for i in range(NCH):
                    nc.tensor.matmul(
                        pq, wqT[:, i, P * j:P * (j + 1)], x0t[:, i, :],
                        start=(i == 0), stop=(i == NCH - 1),
                    )
                nc.vector.tensor_add(
                    q0sb[:, j, :], pq, bq_sb[:, j, None].to_broadcast((P, BPC))
                )

            # blockdiag q0: col m = b*16+h holds q0 of head h, batch b
            q0bd = mp.tile([P, NCH, BPC * H], fp32)
            nc.vector.memset(q0bd, 0.0)
            for j in range(NCH):
                for b in range(BPC):
                    nc.vector.tensor_copy(
                        q0bd[0:64, j, b * H + 2 * j:b * H + 2 * j + 1],
                        q0sb[0:64, j, b:b + 1],
                    )
                    nc.vector.tensor_copy(
                        q0bd[64:128, j, b * H + 2 * j + 1:b * H + 2 * j + 2],
                        q0sb[64:128, j, b:b + 1],
                    )

            # ---- wk native ---------------------------------------------
            wk = wp.tile(BIGSHAPE, fp32, tag="big", name="wk")
            nc.sync.dma_start(
                wk[:, :, :E], wqkv[E:2 * E].rearrange("(j p) c -> p j c", p=P)
            )

            # ---- u = (q0bd^T @ W_k) / 8, c = (q0bd^T @ b_k) / 8 --------
            pu = psU.tile([BPC * H, E], fp32, tag="u")
            for j in range(NCH):
                for s in range(2):
                    nc.tensor.matmul(
                        pu[:, 512 * s:512 * (s + 1)],
                        q0bd[:, j, :],
                        wk[:, j, 512 * s:512 * (s + 1)],
                        start=(j == 0), stop=(j == NCH - 1),
                    )
            usb = mp.tile([BPC * H, E], fp32)
            nc.scalar.mul(usb, pu, 0.125)

            cv = mp.tile([H, BPC], fp32)
            for b in range(BPC):
                pc = psA.tile([H, 1], fp32, tag="sm")
                for j in range(NCH):
                    nc.tensor.matmul(
                        pc, q0bd[:, j, b * H:(b + 1) * H], bk_sb[:, j, None],
                        start=(j == 0), stop=(j == NCH - 1),
                    )
                nc.scalar.mul(cv[:, b:b + 1], pc, 0.125)

            # uT[c_p, ci, m] = usb[m, c]
            uT = mp.tile([P, NCH, BPC * H], f32r)
            for i in range(0, NCH, 2):
                pt = psT.tile([P, 2, P], fp32, tag="tp2")
                nc.tensor.transpose(
                    pt[:, 0, :BPC * H], usb[:, P * i:P * (i + 1)],
                    ident[:BPC * H, :BPC * H],
                )
                nc.tensor.transpose(
                    pt[:, 1, :BPC * H], usb[:, P * (i + 1):P * (i + 2)],
                    ident[:BPC * H, :BPC * H],
                )
                copyback(i, uT[:, i:i + 2, :], pt[:, :, :BPC * H])

            # ---- logits, softmax, wT ----------------------------------
            SEGS = [(0, 512), (512, 512), (1024, 2)]
            wTt = []
            for b in range(BPC):
                lg = lp.tile([H, TP], fp32, tag="lg")
                for (s0, sl) in SEGS:
                    pl = psL.tile([H, 512], fp32, tag="lg")
                    for i in range(NCH):
                        nc.tensor.matmul(
                            pl[:, :sl],
                            uT[:, i, b * H:(b + 1) * H],
                            xaug[b][:, i, s0:s0 + sl],
                            start=(i == 0), stop=(i == NCH - 1),
                        )
                    nc.vector.tensor_add(
                        lg[:, s0:s0 + sl], pl[:, :sl],
                        cv[:, b:b + 1].to_broadcast((H, sl)),
                    )
                nc.vector.memset(lg[:, T:TP], 0.0)
                mx = smp.tile([H, 1], fp32, tag="mx")
                nc.vector.reduce_max(mx, lg[:, :T], axis=AX)
                nmx = smp.tile([H, 1], fp32, tag="nmx")
                nc.scalar.mul(nmx, mx, -1.0)
                nc.scalar.activation(lg[:, :T], lg[:, :T], Exp, bias=nmx, scale=1.0)
                ssum = smp.tile([H, 1], fp32, tag="ssum")
                nc.vector.reduce_sum(ssum, lg[:, :T], axis=AX)
                rs = smp.tile([H, 1], fp32, tag="rs")
                nc.vector.reciprocal(rs, ssum)
                nc.vector.tensor_mul(lg[:, :T], lg[:, :T], rs.to_broadcast((H, T)))

                wT_b = mp.tile([P, 9, H], f32r, tag=f"wT{b}", name=f"wT{b}")
                for k in range(0, 8, 2):
                    pt = psT.tile([P, 2, P], fp32, tag="tp2")
                    nc.tensor.transpose(
                        pt[:, 0, :H], lg[:, P * k:P * (k + 1)], ident[:H, :H]
                    )
                    nc.tensor.transpose(
                        pt[:, 1, :H], lg[:, P * (k + 1):P * (k + 2)], ident[:H, :H]
                    )
                    copyback(k, wT_b[:, k:k + 2, :], pt[:, :, :H])
                pt = psT.tile([P, 2, P], fp32, tag="tp2")
                nc.tensor.transpose(pt[:, 0, :H], lg[:, 1024:1152], ident[:H, :H])
                nc.vector.tensor_copy(wT_b[:, 8, :], pt[:, 0, :H])
                wTt.append(wT_b)

            # ---- z[h, c] = sum_s w[h,s] x_aug[c,s], via staged x_aug^T --
            zT = mp.tile([P, NCH, BPC * H], fp32)
            for b in range(BPC):
                pz0 = psL.tile([H, 512], fp32, tag="lg", name=f"pz0_{b}")
                pz1 = psL.tile([H, 512], fp32, tag="lg", name=f"pz1_{b}")
                for sk in range(9):
                    stq = sp.tile([P, E], f32r, tag="stq", name=f"stq_{b}_{sk}")
                    for ci in range(0, NCH, 2):
                        pt = psT.tile([P, 2, P], fp32, tag="tp2")
                        nc.tensor.transpose(
                            pt[:, 0, :].bitcast(f32r),
                            xaug[b][:, ci, P * sk:P * (sk + 1)],
                            identr,
                        )
                        nc.tensor.transpose(
                            pt[:, 1, :].bitcast(f32r),
                            xaug[b][:, ci + 1, P * sk:P * (sk + 1)],
                            identr,
                        )
                        copyback(ci // 2, stq[:, P * ci:P * (ci + 2)], pt)
                    for q, pz in enumerate((pz0, pz1)):
                        nc.tensor.matmul(
                            pz, wTt[b][:, sk, :],
                            stq[:, 512 * q:512 * (q + 1)],
                            start=(sk == 0), stop=(sk == 8),
                        )
                # z rows -> zT columns
                zrow = sp.tile([P, E], fp32, tag="stq", name=f"zrow_{b}")[:H, :]
                nc.vector.tensor_copy(zrow[:, 0:512], pz0)
                nc.vector.tensor_copy(zrow[:, 512:1024], pz1)
                for i in range(0, NCH, 2):
                    pt = psT.tile([P, 2, P], fp32, tag="tp2")
                    nc.tensor.transpose(
                        pt[:, 0, :H], zrow[:, P * i:P * (i + 1)], ident[:H, :H]
                    )
                    nc.tensor.transpose(
                        pt[:, 1, :H], zrow[:, P * (i + 1):P * (i + 2)],
                        ident[:H, :H],
                    )
                    copyback(i, zT[:, i:i + 2, b * H:(b + 1) * H], pt[:, :, :H])

            # ---- W_v^T then att ---------------------------------------
            wvT = wp.tile(BIGSHAPE, fp32, tag="big", name="wvT")
            build_wT(wvT, wqkv[2 * E:3 * E], "v")

            # head-select masks: mask_j[p, h] = 1 iff h == 2j + p//64
            masks = mp.tile([P, NCH, H], fp32)
            nc.vector.memset(masks, 0.0)
            for j in range(NCH):
                nc.vector.memset(masks[0:64, j, 2 * j:2 * j + 1], 1.0)
                nc.vector.memset(masks[64:128, j, 2 * j + 1:2 * j + 2], 1.0)

            attsel = mp.tile([P, NCH, BPC], fp32)
            for oj in range(NCH):
                pa = psA.tile([P, BPC * H], fp32, tag="sm")
                for ci in range(NCH):
                    nc.tensor.matmul(
                        pa, wvT[:, ci, P * oj:P * (oj + 1)], zT[:, ci, :],
                        start=(ci == 0), stop=(ci == NCH - 1),
                    )
                tmp = smp.tile([P, BPC, H], fp32, tag="asel")
                nc.vector.tensor_mul(
                    tmp, pa.rearrange("p (b h) -> p b h", b=BPC),
                    masks[:, oj, None, :].to_broadcast((P, BPC, H)),
                )
                nc.vector.reduce_sum(attsel[:, oj, :], tmp, axis=AX)
                nc.vector.tensor_add(
                    attsel[:, oj, :], attsel[:, oj, :],
                    bv_sb[:, oj, None].to_broadcast((P, BPC)),
                )

            # ---- W_c^T then out ---------------------------------------
            wcT = wp.tile(BIGSHAPE, fp32, tag="big", name="wcT")
            build_wT(wcT, wc_d, "c")

            osbT = mp.tile([BPC, E], fp32)
            for oj in range(NCH):
                po = psA.tile([P, BPC], fp32, tag="sm")
                for ci in range(NCH):
                    nc.tensor.matmul(
                        po, wcT[:, ci, P * oj:P * (oj + 1)], attsel[:, ci, :],
                        start=(ci == 0), stop=(ci == NCH - 1),
                    )
                ob = smp.tile([P, BPC], fp32, tag="ob")
                nc.vector.tensor_add(
                    ob, po, bc_sb[:, oj, None].to_broadcast((P, BPC))
                )
                pt = psT.tile([P, 2, P], fp32, tag="tp2")
                nc.tensor.transpose(pt[0:BPC, 0, :], ob, ident)
                nc.vector.tensor_copy(
                    osbT[:, P * oj:P * (oj + 1)], pt[0:BPC, 0, :]
                )
            nc.sync.dma_start(out_d[:, :], osbT)

    nc.finalize()
    return nc


def get_nc():
    if "nc" not in _CACHE:
        _CACHE["nc"] = _build()
    return _CACHE["nc"]


def kernel(x, pos_emb, w_qkv, b_qkv, w_c, b_c):
    from concourse.bass_utils import run_bass_kernel_spmd

    nc = get_nc()
    x = np.ascontiguousarray(x, dtype=np.float32)
    in_maps = [
        {
            "x": x[BPC * j:BPC * (j + 1)],
            "pos_emb": np.asarray(pos_emb, dtype=np.float32),
            "w_qkv": np.asarray(w_qkv, dtype=np.float32),
            "b_qkv": np.asarray(b_qkv, dtype=np.float32),
            "w_c": np.asarray(w_c, dtype=np.float32),
            "b_c": np.asarray(b_c, dtype=np.float32),
        }
        for j in range(NCORES)
    ]
    res = run_bass_kernel_spmd(nc, in_maps, list(range(NCORES)))
    return np.concatenate(
        [res.results[j]["out"] for j in range(NCORES)], axis=0
    ).astype(np.float32)
